# revision 1
# baseline (speedup 1.0000x reference)
"""Trainium2 Bass kernel for the Luong-attention module.

Shapes (hardcoded): B=64, T=128, S=1024, IN=1024, OUT=1024.
Sharding: data-parallel over batch across 8 NeuronCores (8 batches/core).
All matmuls run in fp16 (fp32 PSUM accumulation).

Per-core dataflow (contraction dim always on partitions):
  q_projT[i,t]   = sum_o W_attnT[o,i] * QT[o,t]          (once, all 8 batches)
  scores[t,s]    = sum_i q_projT[i,t] * ET[i,s]  (+ mask via K=1 rank-1 mm)
  softmax along s (free axis): negmax -> Exp(bias)+accum_out -> reciprocal
  wT[s,t]        = PE-transpose(w[t,s])                   (8 tiles)
  ctx[t,i]       = sum_s wT[s,t].T * E[s,i]   (wT stationary, N=512 streams)
  ctxT[i,t]      = PE-transpose(ctx[t,i])                 (8 tiles)
  out[t,o]       = tanh(sum_c catT[c,t] * W_outT[c,o] + b_out)
                   with catT k-tiles = [ctxT tiles; QT tiles]

Keeping wT as the ctx stationary means only 8 LDWEIGHTS per batch (vs 64
with E-stationary), which keeps the PE at streaming rate even while input
DMA saturates SBUF write bandwidth.  The out-projection consumes the
decoder-half k-tiles first so the ctxT transposes + PSUM->SBUF casts hide
under PE work.  A short identity-matmul warmup at kernel start burns the
HAM cold window while the first weight DMAs land.
"""

import numpy as np

import concourse.bass as bass
import concourse.mybir as mybir
import concourse.tile as tile
from concourse import bacc
from concourse.bass_utils import run_bass_kernel_spmd
from concourse.masks import make_identity

F16 = mybir.dt.float16
F32 = mybir.dt.float32

N_CORES = 8
B_LOC = 8          # batches per core
T = 128
S = 1024
IN = 1024
OUT = 1024
C = IN + OUT       # concat dim
KO = OUT // 128    # k-tiles over o
KI = IN // 128     # k-tiles over i
KS = S // 128      # k-tiles over s
KC = C // 128      # k-tiles over c
TALL = B_LOC * T   # stacked t across local batches
MASK_NEG = -60000.0
N_WARMUP = 12      # identity matmuls to warm the PE HAM before real work

_CACHED = {}


def _ts(i, sz):
    return slice(i * sz, (i + 1) * sz)


def _build_program(with_bias):
    nc = bacc.Bacc("TRN2", target_bir_lowering=False, debug=False)

    # All big inputs are laid out [.., 128, k, free] so each partition's data
    # is one contiguous chunk in DRAM (128 fat DMA descriptors per load).
    # W_attn and Q are packed per-ko into one tensor: each dma_start costs
    # ~0.6us of HWDGE sequencer time, so one trigger per ko delivering a
    # usable (wat, qt) pair halves the serial trigger cost at kernel start.
    wq = nc.dram_tensor("wq", [128, KO, 2, IN], F16, kind="ExternalInput")
    et = nc.dram_tensor("et", [B_LOC, 128, KI, S], F16, kind="ExternalInput")
    en = nc.dram_tensor("en", [B_LOC, 128, KS, IN], F16, kind="ExternalInput")
    wot = nc.dram_tensor("wot", [128, KC, OUT], F16, kind="ExternalInput")
    msk = nc.dram_tensor("msk", [B_LOC, 1, S], F16, kind="ExternalInput")
    bb = nc.dram_tensor("bb", [1, OUT], F16, kind="ExternalInput")
    w_out = nc.dram_tensor("w_out", [B_LOC, T, S], F16, kind="ExternalOutput")
    att_out = nc.dram_tensor("att_out", [B_LOC, T, OUT], F16, kind="ExternalOutput")

    with tile.TileContext(nc) as tc:
        with (
            tc.tile_pool(name="const", bufs=1) as const_pool,
            tc.tile_pool(name="etp", bufs=2) as et_pool,
            tc.tile_pool(name="enp", bufs=2) as en_pool,
            tc.tile_pool(name="mskp", bufs=2) as msk_pool,
            tc.tile_pool(name="smp", bufs=2) as sm_pool,
            tc.tile_pool(name="statp", bufs=2) as stat_pool,
            tc.tile_pool(name="w16p", bufs=2) as w16_pool,
            tc.tile_pool(name="wtp", bufs=2) as wt_pool,
            tc.tile_pool(name="cxp", bufs=2) as cx_pool,
            tc.tile_pool(name="ctxp", bufs=2) as ctx_pool,
            tc.tile_pool(name="outp", bufs=2) as out_pool,
            tc.tile_pool(name="pssp", bufs=2, space="PSUM") as pss_pool,
            tc.tile_pool(name="pmix", bufs=1, space="PSUM") as pmix_pool,
            tc.tile_pool(name="psop", bufs=1, space="PSUM") as pso_pool,
        ):
            ident = const_pool.tile([128, 128], F16)
            make_identity(nc, ident[:])
            ones = const_pool.tile([1, 128], F16)
            nc.vector.memset(ones[:], 1.0)
            # Pre-load the ACT exp/tanh spline tables (~2.7us) during the
            # DMA-bound head instead of on exp(0)'s critical path.
            actwarm = const_pool.tile([1, 128], F32)
            nc.scalar.activation(actwarm[:], ones[:],
                                 mybir.ActivationFunctionType.Exp)
            if with_bias:
                bb_sb = const_pool.tile([1, OUT], F16)
                nc.sync.dma_start(bb_sb[:], bb[:])

            # Burn the HAM cold window on identity matmuls while the first
            # weight/query DMAs are still in flight.
            warm_ps = pso_pool.tile([128, 128], F32, name="warm", tag="pso")
            for _ in range(N_WARMUP):
                nc.tensor.matmul(warm_ps[:], ident[:], ident[:],
                                 start=True, stop=True)

            # One trigger per ko delivers both wat[ko] and qt[ko].
            wq_sb = const_pool.tile([128, KO, 2, IN], F16)
            for ko in range(KO):
                nc.sync.dma_start(wq_sb[:, ko, :, :], wq[:, ko, :, :])

            qpt_sb = const_pool.tile([128, KI, TALL], F16)

            def load_batch(b, skip_en=False):
                et_sb = et_pool.tile([128, KI, S], F16, name="et")
                nc.sync.dma_start(et_sb[:], et[b])
                msk_sb = msk_pool.tile([1, S], F16, name="msk")
                nc.sync.dma_start(msk_sb[:], msk[b])
                if skip_en:
                    return [et_sb, None, msk_sb]
                en_sb = en_pool.tile([128, KS, IN], F16, name="en")
                nc.sync.dma_start(en_sb[:], en[b])
                return [et_sb, en_sb, msk_sb]

            def load_en(loads, b):
                en_sb = en_pool.tile([128, KS, IN], F16, name="en")
                nc.sync.dma_start(en_sb[:], en[b])
                loads[1] = en_sb

            # ---- Phase 0: q_projT[i, t_all] for all local batches ----
            # mi=0 is DMA-paced (each ko-chunk lands ~1.4us apart); filler
            # identity matmuls between its ko-groups keep the PE HAM from
            # re-throttling during the waits.
            for mi in range(KI):
                psq = pss_pool.tile([128, TALL], F32, name="psq", tag="pss")
                for ko in range(KO):
                    for nh in range(TALL // 512):
                        nc.tensor.matmul(
                            psq[:, _ts(nh, 512)],
                            wq_sb[:, ko, 0, _ts(mi, 128)],
                            wq_sb[:, ko, 1, _ts(nh, 512)],
                            start=(ko == 0),
                            stop=(ko == KO - 1),
                        )
                    if mi == 0:
                        for _ in range(5):
                            nc.tensor.matmul(warm_ps[:], ident[:], ident[:],
                                             start=True, stop=True)
                nc.vector.tensor_copy(qpt_sb[:, mi, :], psq[:])

            # DMA order: et0/en0, et1, wot, en1 — each load lands just before
            # its first consumer, with wot ahead of en1 (out(0) precedes
            # ctx(1)).
            first_loads = load_batch(0)
            second_loads = load_batch(1, skip_en=True)

            wot_sb = const_pool.tile([128, KC, OUT], F16)
            nc.sync.dma_start(wot_sb[:], wot[:])
            load_en(second_loads, 1)

            def scores_mms(b, loads):
                et_sb, _, msk_sb = loads
                pss = pss_pool.tile([128, S], F32, name="pss")
                for nh in range(S // 512):
                    nc.tensor.matmul(
                        pss[:, _ts(nh, 512)],
                        ones[:1, :],
                        msk_sb[:1, _ts(nh, 512)],
                        start=True,
                        stop=False,
                    )
                for ki in range(KI):
                    for nh in range(S // 512):
                        nc.tensor.matmul(
                            pss[:, _ts(nh, 512)],
                            qpt_sb[:, ki, _ts(b, T)],
                            et_sb[:, ki, _ts(nh, 512)],
                            start=False,
                            stop=(ki == KI - 1),
                        )
                return pss

            def softmax_front(b, pss):
                negmx = stat_pool.tile([128, 1], F32, name="negmx")
                nc.vector.reduce_max(
                    negmx[:], pss[:], axis=mybir.AxisListType.X, negate=True
                )
                ew = sm_pool.tile([128, S], F16, name="ew")
                ssum = stat_pool.tile([128, 1], F32, name="ssum")
                nc.scalar.activation(
                    ew[:],
                    pss[:],
                    mybir.ActivationFunctionType.Exp,
                    bias=negmx[:],
                    scale=1.0,
                    accum_out=ssum[:],
                )
                return ew, ssum

            def softmax_back(b, ew, ssum):
                rs = stat_pool.tile([128, 1], F32, name="rs")
                nc.vector.reciprocal(rs[:], ssum[:])
                w16 = w16_pool.tile([128, S], F16, name="w16")
                nc.vector.tensor_scalar_mul(w16[:], ew[:], rs[:])
                nc.scalar.dma_start(w_out[b], w16[:])
                return rs

            def transp_w(w16):
                # wT[s, t] via PE transpose (8 tiles into one PSUM bank)
                pst = pmix_pool.tile([128, KS, T], F16, name="pst", tag="mix")
                for st in range(KS):
                    nc.tensor.matmul(
                        pst[:, st, :],
                        w16[:, _ts(st, 128)],
                        ident[:],
                        is_transpose=True,
                        start=(st == 0),
                        stop=(st == KS - 1),
                    )
                wt_sb = wt_pool.tile([128, KS, T], F16, name="wt")
                nc.vector.tensor_copy(wt_sb[:, : KS // 2, :], pst[:, : KS // 2, :])
                nc.vector.tensor_copy(wt_sb[:, KS // 2 :, :], pst[:, KS // 2 :, :])
                return wt_sb

            def ctx_mms(en_sb, wt_sb, rs):
                # ctxu[t, i] = sum_s ew[t,s] E[s,i]: ewT tiles stationary (8
                # LDWEIGHTS/batch), E streams at N=512.  The softmax 1/sum is
                # a per-t (= per-partition) factor here, so it folds into the
                # PSUM->SBUF cast as a tensor_scalar multiply.
                psc = pmix_pool.tile([128, IN], F32, name="psc", tag="mix")
                for ks in range(KS):
                    for nh in range(IN // 512):
                        nc.tensor.matmul(
                            psc[:, _ts(nh, 512)],
                            wt_sb[:, ks, :],
                            en_sb[:, ks, _ts(nh, 512)],
                            start=(ks == 0),
                            stop=(ks == KS - 1),
                        )
                cx16 = cx_pool.tile([128, IN], F16, name="cx16")
                nc.vector.tensor_scalar_mul(
                    cx16[:, : IN // 2], psc[:, : IN // 2], rs[:])
                nc.vector.tensor_scalar_mul(
                    cx16[:, IN // 2 :], psc[:, IN // 2 :], rs[:])
                return cx16

            def out_and_store(b, cx16):
                # out[t, o] = tanh(catT.T @ W_outT + b_out), catT k-tiles =
                # [ctxT (from transposing cx16); qT].  Decoder-half k-tiles
                # stream first so the ctxT transposes + casts hide under them.
                nh_all = list(range(OUT // 512))
                pso = pso_pool.tile([128, OUT], F32, name="pso", tag="pso")
                if with_bias:
                    for nh in nh_all:
                        nc.tensor.matmul(
                            pso[:, _ts(nh, 512)],
                            ones[:1, :],
                            bb_sb[:1, _ts(nh, 512)],
                            start=True,
                            stop=False,
                        )

                def qt_tiles(rng):
                    for j, kq in enumerate(rng):
                        for nh in nh_all:
                            nc.tensor.matmul(
                                pso[:, _ts(nh, 512)],
                                wq_sb[:, kq, 1, _ts(b, T)],
                                wot_sb[:, KI + kq, _ts(nh, 512)],
                                start=(not with_bias and kq == 0),
                                stop=False,
                            )

                qt_tiles(range(0, KO // 2))

                # ctxT[i, t] via PE transpose of cx16, mid-stream
                pct = pmix_pool.tile([128, KI, T], F16, name="pct", tag="mix")
                for j in range(KI):
                    nc.tensor.matmul(
                        pct[:, j, :],
                        cx16[:, _ts(j, 128)],
                        ident[:],
                        is_transpose=True,
                        start=(j == 0),
                        stop=(j == KI - 1),
                    )

                qt_tiles(range(KO // 2, KO))

                ctxt_sb = ctx_pool.tile([128, KI, T], F16, name="ctxT")
                nc.vector.tensor_copy(ctxt_sb[:, : KI // 2, :], pct[:, : KI // 2, :])
                nc.vector.tensor_copy(ctxt_sb[:, KI // 2 :, :], pct[:, KI // 2 :, :])

                for kc in range(KI):
                    for nh in nh_all:
                        nc.tensor.matmul(
                            pso[:, _ts(nh, 512)],
                            ctxt_sb[:, kc, :],
                            wot_sb[:, kc, _ts(nh, 512)],
                            start=False,
                            stop=(kc == KI - 1),
                        )

                if b == B_LOC - 1:
                    # last batch: split quarters so tanh/DMA pipeline the tail
                    for q in range(4):
                        osb = out_pool.tile([128, 256], F16, name=f"osb{q}",
                                            tag="out_sb")
                        nc.scalar.activation(
                            osb[:], pso[:, _ts(q, 256)],
                            mybir.ActivationFunctionType.Tanh,
                        )
                        nc.scalar.dma_start(att_out[b][:, _ts(q, 256)], osb[:])
                else:
                    osb = out_pool.tile([128, OUT], F16, name="osb",
                                        tag="out_sb")
                    nc.scalar.activation(
                        osb[:], pso[:], mybir.ActivationFunctionType.Tanh
                    )
                    nc.scalar.dma_start(att_out[b], osb[:])

            # ---- Pipelined batch loop ----
            # Per-engine emission order matters: each engine executes its
            # stream in order.  PE: transp_w(b-1), scores(b), ctx(b-1),
            # out(b-1) [qt-half, ctxT transposes, qt-half, ctx-half].
            # DVE: wt casts(b-1), negmax(b), cx casts(b-1), pct casts(b-1),
            # recip(b), w16(b).
            pending = None
            loads, next_loads = first_loads, second_loads
            for b in range(B_LOC):
                if b == 1:
                    # scores(1) ahead of transp_w(0): covers the softmax(0)
                    # DVE/ACT chain that ew(0) (and thus transp_w(0)) waits
                    # on, so the PE never idles at the qproj->loop seam.
                    pss = scores_mms(b, loads)
                    pb, pew, prs, pen = pending
                    wt_sb = transp_w(pew)
                    ew, ssum = softmax_front(b, pss)
                else:
                    if pending is not None:
                        pb, pew, prs, pen = pending
                        wt_sb = transp_w(pew)
                    pss = scores_mms(b, loads)
                    ew, ssum = softmax_front(b, pss)
                if pending is not None:
                    cx16 = ctx_mms(pen, wt_sb, prs)
                rs = softmax_back(b, ew, ssum)
                if pending is not None:
                    out_and_store(pb, cx16)
                pending = (b, ew, rs, loads[1])
                loads = next_loads
                next_loads = load_batch(b + 2) if b + 2 < B_LOC else None
            pb, pew, prs, pen = pending
            wt_sb = transp_w(pew)
            cx16 = ctx_mms(pen, wt_sb, prs)
            out_and_store(pb, cx16)

    nc.compile()
    return nc


def _get_nc(with_bias):
    if with_bias not in _CACHED:
        _CACHED[with_bias] = _build_program(with_bias)
    return _CACHED[with_bias]


def _prep_inputs(decoder_output, encoder_outputs, encoder_padding_mask,
                 W_attn, W_out, b_out):
    f16 = np.float16
    wat_h = W_attn.T.reshape(KO, 128, IN).swapaxes(0, 1).astype(f16)
    wot_h = W_out.T.reshape(KC, 128, OUT).swapaxes(0, 1).astype(f16)
    bb_h = b_out.reshape(1, OUT).astype(f16)

    in_maps = []
    for c in range(N_CORES):
        sl = slice(c * B_LOC, (c + 1) * B_LOC)
        dec = decoder_output[sl]          # [8, T, OUT] f32
        enc = encoder_outputs[sl]         # [8, S, IN] f32
        m = encoder_padding_mask[sl]      # [8, S] bool
        qt_h = (
            dec.transpose(2, 0, 1).reshape(KO, 128, TALL)
            .swapaxes(0, 1).astype(f16)
        )
        wq_h = np.stack((wat_h, qt_h), axis=2)
        et_h = (
            enc.transpose(0, 2, 1).reshape(B_LOC, KI, 128, S)
            .swapaxes(1, 2).astype(f16)
        )
        en_h = (
            enc.reshape(B_LOC, KS, 128, IN).swapaxes(1, 2).astype(f16)
        )
        msk_h = np.where(m, np.float16(MASK_NEG), np.float16(0.0)).reshape(
            B_LOC, 1, S
        )
        in_maps.append(
            {
                "wq": wq_h,
                "et": et_h,
                "en": en_h,
                "wot": wot_h,
                "msk": msk_h,
                "bb": bb_h,
            }
        )
    return in_maps


def kernel(decoder_output, encoder_outputs, encoder_padding_mask,
           W_attn, W_out, b_out, _trace=False, _tmpdir=None):
    decoder_output = np.asarray(decoder_output, dtype=np.float32)
    encoder_outputs = np.asarray(encoder_outputs, dtype=np.float32)
    encoder_padding_mask = np.asarray(encoder_padding_mask)
    W_attn = np.asarray(W_attn, dtype=np.float32)
    W_out = np.asarray(W_out, dtype=np.float32)
    b_out = np.asarray(b_out, dtype=np.float32)

    with_bias = bool(np.any(b_out != 0))
    nc = _get_nc(with_bias)
    in_maps = _prep_inputs(
        decoder_output, encoder_outputs, encoder_padding_mask,
        W_attn, W_out, b_out,
    )
    kw = {}
    if _trace:
        kw = {"trace": True, "tmpdir": _tmpdir}
    res = run_bass_kernel_spmd(nc, in_maps, core_ids=list(range(N_CORES)), **kw)
    attn_outputs = np.concatenate(
        [r["att_out"] for r in res.results], axis=0
    ).astype(np.float32)
    attn_weights = np.concatenate(
        [r["w_out"] for r in res.results], axis=0
    ).astype(np.float32)
    kernel._last_results = res
    return attn_outputs, attn_weights

